# revision 23
# baseline (speedup 1.0000x reference)
"""BitLinear (fake-quant straight-through) Trainium2 kernel.

Math (per the reference nn module):
  dqx = round(x * s_x) / s_x       s_x = 127 / clip(rowabsmax(x), 1e-5)  (per token)
  dqw = clip(round(w * s_w), -1, 1) / s_w   s_w = 1 / clip(mean|w|, 1e-5) (per tensor)
  out = dqx @ dqw.T + bias

Design:
  * round(x*s_x) is an integer in [-127,127] and the ternary weight is in
    {-1,0,1}; both are EXACT in bf16 and the matmul accumulates exactly in
    fp32 PSUM, so the heavy matmul runs at full bf16 PE rate.  round() is
    the fp32-RNE magic-constant trick (v + 1.5*2^23) - 1.5*2^23.
  * Host-side input prep (all O(S*K) or O(N*K), ~0.1% of the matmul
    FLOPs): weight is ternary-quantized + transposed bit-exactly with the
    reference's rounding; per-token scales ss/fs come from the exact f32
    row absmax; x ships as fp16 and out returns as bf16, spending the
    validated ~3.8e-3 rel-err (gate 2e-2) to halve HBM traffic.
  * Per 512-token quad: fp16 x load (scalar/ACT HWDGE ring) -> gpsimd
    round (fp16 in, f32 out; the f32->bf16 write path on gpsimd is ~50x
    slow, never use it) -> ACT affine -MAGIC + bf16 cast (Sterbenz-exact)
    -> one xbar transpose (sync/SP ring) -> 64 back-to-back 512-wide bf16
    matmuls -> DVE scalar_tensor_tensor fused evac bf16(psum*fs + bias)
    -> SWDGE store (gpsimd ring).  Every pipeline stage owns one engine
    and one DMA ring, so stages only queue behind themselves.
  * Tile's xbar-hang workaround makes each DMA transpose wait for ALL
    earlier-scheduled DMA copies; add_dep_helper pins load(q) after
    transpose(q-2) in the schedule so transposes never stall on far-future
    loads (worth ~25 us end-to-end).

Sharding: data parallel over batch; core i computes batch element i with
the full weight.  No collectives; the host scatters x / gathers out.
"""

import numpy as np

from concourse import bacc, bass, mybir, tile
from concourse.bass_utils import run_bass_kernel_spmd
from concourse.tile_rust import add_dep_helper

F32 = mybir.dt.float32
FP16 = mybir.dt.float16
BF16 = mybir.dt.bfloat16
ALU = mybir.AluOpType
ACTF = mybir.ActivationFunctionType

MAGIC = 12582912.0  # 1.5 * 2**23: fp32 RNE round-to-integer constant
EPS = 1e-05

B, S, K, N = 8, 4096, 1024, 1024
N_CORES = 8
QS = 4  # token tiles per quad


def build(s_tokens=S, k=K, n=N):
    nc = bacc.Bacc("TRN2", target_bir_lowering=False, debug=False)

    KT = k // 128
    NT = n // 128
    NH = n // 512
    NQ = s_tokens // (128 * QS)
    NC = NQ * QS  # scale columns

    x_d = nc.dram_tensor("x", [s_tokens, k], FP16, kind="ExternalInput").ap()
    qwt_d = nc.dram_tensor("qwt", [128, NT, KT, 128], BF16, kind="ExternalInput").ap()
    bias_d = nc.dram_tensor("biasb", [128, n], F32, kind="ExternalInput").ap()
    # scales[p, 0:NC] = ss per token, scales[p, NC:2NC] = fs per token
    scales_d = nc.dram_tensor("scales", [128, 2 * NC], F32, kind="ExternalInput").ap()
    out_d = nc.dram_tensor("out", [s_tokens, n], BF16, kind="ExternalOutput").ap()

    x_q = x_d.rearrange("(q s p) k -> q p s k", s=QS, p=128)
    out_q = out_d.rearrange("(q s p) n -> q p s n", s=QS, p=128)

    HS = QS // 2  # token tiles per half-quad (output/store granularity)

    with tile.TileContext(nc) as tc:
        with (
            tc.tile_pool(name="static", bufs=1) as static,
            tc.tile_pool(name="xpool", bufs=4) as xpool,
            tc.tile_pool(name="x0pool", bufs=QS) as x0pool,
            tc.tile_pool(name="y0pool", bufs=2) as y0pool,
            tc.tile_pool(name="q0pool", bufs=2) as q0pool,
            tc.tile_pool(name="qt0pool", bufs=QS) as qt0pool,
            tc.tile_pool(name="ypool", bufs=2) as ypool,
            tc.tile_pool(name="qpool", bufs=3) as qpool,
            tc.tile_pool(name="qtpool", bufs=3) as qtpool,
            tc.tile_pool(name="opool", bufs=6) as opool,
            tc.tile_pool(name="psum", bufs=3, space="PSUM") as psum_pool,
        ):
            # scales (tiny) + qwT alone on the SWDGE ring so qwT lands as
            # early as the 8-core HBM burst allows; bias rides the scalar
            # ring but is PINNED after quad 0's first transpose so the
            # xbar-hang serialization can't gate that transpose on it.
            scales = static.tile([128, 2 * NC], F32)
            nc.gpsimd.dma_start(scales[:], scales_d[:])
            qwT = static.tile([128, NT, KT, 128], BF16)
            nc.gpsimd.dma_start(qwT[:], qwt_d[:])
            bias_sb = static.tile([128, n], F32)

            def evac(outs, fcol, col, ps_list):
                # fused evac: outs = bf16(psum * fs[s] + bias)
                for h in range(NH):
                    nc.vector.scalar_tensor_tensor(
                        outs[:, fcol, h * 512:(h + 1) * 512],
                        ps_list[h][:],
                        scales[:, NC + col:NC + col + 1],
                        bias_sb[:, h * 512:(h + 1) * 512],
                        ALU.mult,
                        ALU.add,
                    )

            def mms(qxT_s, ps_list):
                for kt in range(KT):
                    for h in range(NH):
                        nc.tensor.matmul(
                            ps_list[h][:],
                            qxT_s[:, kt, :],
                            qwT[:, 4 * h:4 * h + 4, kt, :],
                            start=(kt == 0),
                            stop=(kt == KT - 1),
                        )

            transp_insts = []

            # ---- quad 0: per-s-tile pipeline (separate tiles => true
            # slice-granular deps) so the first matmul only needs one 256 KB
            # x slice + the qwT load instead of the whole quad chain ----
            outs_u = [opool.tile([128, HS, n], BF16, name="outs_u")
                      for _ in range(2)]
            for s in range(QS):
                x0 = x0pool.tile([128, 1, k], FP16, name="x_q0")
                nc.scalar.dma_start(x0[:], x_q[0][:, s:s + 1, :])
                y0 = y0pool.tile([128, 1, k], F32, name="y_q0")
                nc.gpsimd.tensor_scalar(
                    y0[:, 0, :], x0[:, 0, :], scales[:, s:s + 1], MAGIC,
                    ALU.mult, ALU.add,
                )
                qx0 = q0pool.tile([128, 1, k], BF16, name="qx_q0")
                nc.scalar.activation(qx0[:], y0[:], ACTF.Copy, bias=-MAGIC)
                qxT0 = qt0pool.tile([128, 1, KT, 128], BF16, name="qxT_q0")
                t_inst = nc.sync.dma_start_transpose(qxT0[:], qx0[:])
                if s == 0:
                    bias_inst = nc.scalar.dma_start(bias_sb[:], bias_d[:])
                    add_dep_helper(
                        bias_inst.ins, t_inst.ins, sync=False,
                        reason="bias load after first transpose in schedule",
                    )
                if s == QS - 1:
                    transp_insts.append(t_inst)
                ps_list = [
                    psum_pool.tile([128, 512], F32, name=f"ps{h}", tag=f"ps{h}")
                    for h in range(NH)
                ]
                mms(qxT0[:, 0], ps_list)
                evac(outs_u[s // HS], s % HS, s, ps_list)
                if s % HS == HS - 1:
                    nc.gpsimd.dma_start(
                        out_q[0][:, s - HS + 1:s + 1, :], outs_u[s // HS][:]
                    )

            # ---- quads 1..NQ-1: whole-quad pipeline ----
            for q in range(1, NQ):
                x_s = xpool.tile([128, QS, k], FP16, name="x_s")
                load_inst = nc.scalar.dma_start(x_s[:], x_q[q])
                if q >= 2:
                    # schedule-order pin: Tile's xbar-hang workaround makes
                    # every DMA transpose wait for ALL earlier-scheduled DMA
                    # copies; without this pin the scheduler hoists far-
                    # future x loads ahead of transpose(q-2), which then
                    # stalls on them.
                    add_dep_helper(
                        load_inst.ins, transp_insts[q - 2].ins, sync=False,
                        reason="keep load(q) after transpose(q-2) in schedule",
                    )

                # round(x*s_x) via magic constant (gpsimd: fp16 in, f32 out)
                y_s = ypool.tile([128, QS, k], F32, name="y_s")
                for s in range(QS):
                    col = q * QS + s
                    nc.gpsimd.tensor_scalar(
                        y_s[:, s, :], x_s[:, s, :],
                        scales[:, col:col + 1], MAGIC,
                        ALU.mult, ALU.add,
                    )
                # -MAGIC + bf16 cast on ACT (Sterbenz-exact affine)
                qx = qpool.tile([128, QS, k], BF16, name="qx")
                nc.scalar.activation(qx[:], y_s[:], ACTF.Copy, bias=-MAGIC)

                # one xbar transpose for the whole quad
                qxT = qtpool.tile([128, QS, KT, 128], BF16, name="qxT")
                transp_insts.append(nc.sync.dma_start_transpose(qxT[:], qx[:]))

                outs_u = [opool.tile([128, HS, n], BF16, name="outs_u")
                          for _ in range(2)]
                for s in range(QS):
                    col = q * QS + s
                    ps_list = [
                        psum_pool.tile([128, 512], F32, name=f"ps{h}", tag=f"ps{h}")
                        for h in range(NH)
                    ]
                    mms(qxT[:, s], ps_list)
                    evac(outs_u[s // HS], s % HS, col, ps_list)
                    # store each half as soon as its two evacs land (halves
                    # the tail and smooths the store stream)
                    if s % HS == HS - 1:
                        nc.gpsimd.dma_start(
                            out_q[q][:, s - HS + 1:s + 1, :], outs_u[s // HS][:]
                        )

    nc.compile()
    return nc


def host_weight(weight):
    import ml_dtypes

    w = np.ascontiguousarray(weight, dtype=np.float32)
    try:
        import jax
        import jax.numpy as jnp

        with jax.default_device(jax.devices("cpu")[0]):
            mean_abs = np.float32(
                jax.device_get(jnp.mean(jnp.abs(jnp.asarray(w, dtype=jnp.float32))))
            )
    except Exception:
        mean_abs = np.float32(np.mean(np.abs(w), dtype=np.float32))
    mean_c = np.maximum(mean_abs, np.float32(EPS))
    sw = np.float32(1.0) / mean_c
    tern = np.clip(np.rint(w * sw), -1.0, 1.0).astype(ml_dtypes.bfloat16)
    NT, KT = N // 128, K // 128
    qwt = np.ascontiguousarray(
        tern.reshape(NT, 128, KT, 128).transpose(3, 0, 2, 1)
    )
    wdiv = np.float32(1.0) / sw
    k1 = wdiv / np.float32(127.0)
    return qwt, k1


def host_scales(x_core, k1):
    """Per-token ss/fs from the exact f32 x (matches reference absmax)."""
    cc = np.maximum(
        np.abs(x_core).max(axis=1), np.float32(EPS)
    ).astype(np.float32)                       # [s_tokens]
    ssv = np.float32(127.0) / cc               # one division, like the reference
    fsv = cc * np.float32(k1)
    NQ = x_core.shape[0] // 512
    # token t = q*512 + s*128 + p  ->  scales[p, q*QS + s]
    ss_t = ssv.reshape(NQ * QS, 128).T         # [128, NQ*QS]
    fs_t = fsv.reshape(NQ * QS, 128).T
    return np.ascontiguousarray(
        np.concatenate([ss_t, fs_t], axis=1), dtype=np.float32
    )


def make_in_maps(x, weight, bias):
    x = np.ascontiguousarray(x, dtype=np.float32)
    x16 = x.astype(np.float16)
    bias = np.ascontiguousarray(bias, dtype=np.float32)
    qwt, k1 = host_weight(weight)
    biasb = np.tile(bias[None, :], (128, 1)).copy()
    return [
        {
            "x": x16[i],
            "qwt": qwt,
            "biasb": biasb,
            "scales": host_scales(x[i], k1),
        }
        for i in range(N_CORES)
    ]


_NC_CACHE = {}


def _get_nc():
    if "nc" not in _NC_CACHE:
        _NC_CACHE["nc"] = build()
    return _NC_CACHE["nc"]


def kernel(x, weight, bias, **kwargs):
    nc = _get_nc()
    in_maps = make_in_maps(x, weight, bias)
    last_err = None
    for _attempt in range(3):
        try:
            res = run_bass_kernel_spmd(nc, in_maps, list(range(N_CORES)))
            return np.stack(
                [
                    np.asarray(res.results[i]["out"]).astype(np.float32)
                    for i in range(N_CORES)
                ],
                axis=0,
            )
        except Exception as e:  # transient NRT device errors: retry
            last_err = e
    raise last_err
